# revision 13
# baseline (speedup 1.0000x reference)
"""Int4 group-quantized dense MLP matmul on 8 Trainium2 NeuronCores.

Computes out[b,s,n] = sum_k x[b,s,k] * W[n,k] where W is dequantized from
packed int4 (two nibbles per byte, per-128-group fp16 scales).

Strategy (tensor-parallel over N):
  - N=11008 output features sharded 1376 per core across 8 cores.
  - Host does LAYOUT ONLY: transpose x to k-major (with a per-k-tile
    nibble-parity permutation so device-side dequant never needs a
    transpose), transpose/duplicate the packed weight bytes, replicate
    scales. All arithmetic (nibble extract, -8, x scale, matmul) runs on
    device.
  - Device per core: dequantize W^T into SBUF-resident fp16 tiles
    [128 k x 1376 n] (lo nibbles -> partitions 0-63, hi -> 64-127), then
    for each 128-token tile accumulate 32 k-tile matmuls into PSUM fp32
    and write fp16 output rows.
"""

import numpy as np

B, S, K, N = 4, 2048, 4096, 11008
T = B * S                      # 8192 tokens
P = 128                        # partitions
KT = K // P                    # 32 k-tiles (each is one quant group)
NCORES = 8
NC_N = N // NCORES             # 1376 features per core
TOK_CHUNK = 256                # tokens per x DMA chunk
MM_FREE = 512                  # max moving free dim per matmul (PSUM bank)
# Last FP8_KT k-groups run as fp8e4 DoubleRow pairs (2 groups per matmul,
# 2x PE rate); the rest stay fp16. Bounded by the 2e-2 rel-err gate:
# measured 3.8% rel err if all 32 groups are fp8, ~1.9% at 8/32.
FP8_KT = 8
FP16_KT = KT - FP8_KT


# ---------------------------------------------------------------------------
# walrus in this container accepts only ONE sync wait per instruction;
# split extras onto same-engine NoOps placed immediately before.
def _legalize_multi_waits(nc, max_waits=1):
    from concourse import mybir

    n_fixed = 0
    for f in nc.m.functions:
        for bb in f.blocks:
            insts = bb.instructions
            i = 0
            while i < len(insts):
                inst = insts[i]
                si = inst.sync_info
                if si is not None and si.on_wait and len(si.on_wait) > max_waits:
                    waits = list(si.on_wait)
                    extra, keep = waits[:-max_waits], waits[-max_waits:]
                    chain = []
                    for j in range(0, len(extra), max_waits):
                        chunk = extra[j : j + max_waits]
                        chain.append(
                            mybir.InstNoOp(
                                name=f"{inst.name}-waitsplit-{j}",
                                engine=inst.engine,
                                bass_nofuse=True,
                                sync_info=mybir.SyncInfo(on_wait=chunk, on_update=[]),
                            )
                        )
                    si.on_wait = keep
                    for k, nop in enumerate(chain):
                        insts.insert(i + k, nop)
                    i += len(chain)
                    n_fixed += 1
                i += 1
    return n_fixed


def _install_ntff_shim():
    """Make trace=True work: register the NTFF profile hook that the agent
    image's antenv lacks, and keep artifacts local."""
    import sys, types

    try:
        import antenv.axon_hooks  # noqa: F401

        return
    except ImportError:
        pass
    try:
        from trn_agent_boot.trn_boot import _ntff_profile_via_ctypes

        hook = _ntff_profile_via_ctypes("/opt/axon/libaxon_pjrt.so")
    except Exception:
        hook = None
    mod = types.ModuleType("antenv.axon_hooks")
    mod.get_axon_ntff_profile_hook = lambda: hook
    mod.set_axon_ntff_profile_hook = lambda h: None
    sys.modules["antenv.axon_hooks"] = mod

    import concourse.bass_utils as bu

    bu.upload_artifacts = lambda tmpdir: "local://" + str(tmpdir)


# ---------------------------------------------------------------------------
def build_nc(t=T, k=K, nc_n=NC_N, tok_chunk=TOK_CHUNK):
    """Build the per-core Bass program (same NEFF on all cores; per-core
    inputs differ). Inputs: xTp [k, t] fp16, wpk [kt, 128, nc_n] u8,
    scl [kt, 128, nc_n] fp16. Output: out [t, nc_n] fp16."""
    import concourse.bass as bass
    import concourse.tile as tile
    from concourse import mybir

    kt_n = k // P
    assert t % tok_chunk == 0 and tok_chunk % P == 0
    n_splits = [
        (n0, min(MM_FREE, nc_n - n0)) for n0 in range(0, nc_n, MM_FREE)
    ]

    nc = bass.Bass()
    # const AP for the ACT bias (-8): same pattern as Bass.__init__ consts
    _c = nc.alloc_sbuf_tensor("const-float32-m8", [P, 1], mybir.dt.float32)
    nc.gpsimd.memset(_c.ap(), -8.0)
    nc.const_aps.aps[(mybir.dt.float32, -8.0)] = _c.ap()
    nc.all_engine_barrier()

    xTp = nc.declare_dram_parameter("xTp", [k, t], mybir.dt.float16, isOutput=False)
    wpk = nc.declare_dram_parameter(
        "wpk", [kt_n, P, nc_n], mybir.dt.uint8, isOutput=False
    )
    # scales ship as one row per k-group (2.75 KB vs 352 KB replicated); the
    # 128-partition replication happens on the PE (ones x srow -> PSUM),
    # keeping the prologue's DMA-engine bottleneck free of scale bytes.
    scl = nc.declare_dram_parameter(
        "scl", [kt_n, 1, nc_n], mybir.dt.float16, isOutput=False
    )
    out = nc.declare_dram_parameter("out", [t, nc_n], mybir.dt.float16, isOutput=True)

    fp16_kt = kt_n - FP8_KT
    n_pairs = FP8_KT // 2

    with tile.TileContext(nc) as tc:
        with (
            tc.tile_pool(name="wt", bufs=1) as wt_pool,
            tc.tile_pool(name="wt8", bufs=1) as wt8_pool,
            tc.tile_pool(name="tmp16", bufs=3) as tmp_pool,
            tc.tile_pool(name="wsb", bufs=3) as wsb_pool,
            tc.tile_pool(name="ssb", bufs=3) as ssb_pool,
            tc.tile_pool(name="xt", bufs=3) as x_pool,
            tc.tile_pool(name="x8", bufs=3) as x8_pool,
            tc.tile_pool(name="osb", bufs=3) as out_pool,
            tc.tile_pool(name="ones", bufs=1) as ones_pool,
            tc.tile_pool(name="psum", bufs=2, space="PSUM") as psum_pool,
            tc.tile_pool(name="psb", bufs=2, space="PSUM") as psb_pool,
        ):
            xv = xTp.rearrange("(kt p) t -> p kt t", p=P)
            ksplit = 4 if kt_n % 4 == 0 else 1
            ktn_per = kt_n // ksplit
            # fp8 groups must cover whole x DMA sub-tiles
            assert fp16_kt % ktn_per == 0 and FP8_KT % 2 == 0
            s_fp8 = fp16_kt // ktn_per  # first fp8 sub-tile index

            def load_x_chunk(c0):
                """x chunk as `ksplit` sub-tiles on separate DMA queues so the
                first k-tiles land early. Sub-tiles covering fp8 k-groups are
                additionally converted to an fp8e4 copy (DVE) for DoubleRow."""
                subs = []
                for s in range(ksplit):
                    xs = x_pool.tile(
                        [P, ktn_per, tok_chunk], mybir.dt.float16,
                        tag=f"xt{s}", name=f"xt{s}_{c0}",
                    )
                    nc.sync.dma_start(
                        xs[:],
                        xv[:, s * ktn_per : (s + 1) * ktn_per, c0 : c0 + tok_chunk],
                    )
                    subs.append(xs)
                x8c = x8_pool.tile(
                    [P, FP8_KT, tok_chunk], mybir.dt.float8e4,
                    tag="x8", name=f"x8_{c0}",
                )
                for s in range(s_fp8, ksplit):
                    off = s * ktn_per - fp16_kt
                    nc.vector.tensor_scalar(
                        x8c[:, off : off + ktn_per, :], subs[s][:], 1.0, None,
                        mybir.AluOpType.mult,
                    )
                return subs, x8c

            def alloc_psums(ts_abs):
                return [
                    psum_pool.tile(
                        [P, MM_FREE], mybir.dt.float32,
                        tag=f"ps{j}", name=f"ps{j}_{ts_abs}",
                    )
                    for j in range(len(n_splits))
                ]

            def emit_mms(psums, xsubs, ts, kt):
                lhsT = xsubs[kt // ktn_per][
                    :, kt % ktn_per, ts * P : (ts + 1) * P
                ]
                for j, (n0, w) in enumerate(n_splits):
                    nc.tensor.matmul(
                        psums[j][:, :w],
                        lhsT,
                        wt_tiles[kt][:, n0 : n0 + w],
                        start=(kt == 0),
                        stop=(FP8_KT == 0 and kt == kt_n - 1),
                    )

            def emit_mms_fp8(psums, x8c, ts, pr):
                lhsT = x8c[:, 2 * pr : 2 * pr + 2, ts * P : (ts + 1) * P]
                for j, (n0, w) in enumerate(n_splits):
                    nc.tensor.matmul(
                        psums[j][:, :w],
                        lhsT,
                        wt8_tiles[pr][:, :, n0 : n0 + w],
                        start=False,
                        stop=(pr == n_pairs - 1),
                        perf_mode=mybir.MatmulPerfMode.DoubleRow,
                    )

            def emit_tail(psums, r0):
                osb = out_pool.tile(
                    [P, nc_n], mybir.dt.float16, tag="osb", name=f"osb{r0}"
                )
                for j, (n0, w) in enumerate(n_splits):
                    nc.scalar.copy(osb[:, n0 : n0 + w], psums[j][:, :w])
                nc.sync.dma_start(out[r0 : r0 + P, :], osb[:])

            # ---- dequant prologue: W^T tiles resident in SBUF.
            # The first two 128-token tiles' matmuls are interleaved
            # kt-outer so the in-order PE banks work while W tiles are
            # still being produced (prologue is DMA-rate-bound).
            xt_tiles = {}
            wt_tiles = []
            wt8_tiles = [None] * n_pairs
            # all-ones stationary column for the scale-broadcast matmuls
            ones = ones_pool.tile([1, P], mybir.dt.float16, tag="ones")
            nc.vector.memset(ones[:], 1.0)
            # per-partition nibble shift: 0 for the lo half, 4 for the hi half
            shv = ones_pool.tile([P, 1], mybir.dt.uint32, tag="shv")
            nc.vector.memset(shv[0:64, :], 0)
            nc.vector.memset(shv[64:P, :], 4)
            for kt in range(kt_n):
                wsb = wsb_pool.tile([P, nc_n], mybir.dt.uint8, tag="wsb")
                nc.sync.dma_start(wsb[:], wpk[kt])
                srow = ssb_pool.tile([1, nc_n], mybir.dt.float16, tag="ssb")
                nc.sync.dma_start(srow[:], scl[kt])
                # lo nibbles in partitions 0-63, hi in 64-127 (host duplicated
                # the bytes into both halves; DVE lanes stay in-partition).
                # Word-wise nibble extraction: process 4 bytes per lane-cycle
                # via a u32 view; the 0x0F0F0F0F mask clears cross-byte bits.
                w32 = wsb[:].bitcast(mybir.dt.uint32)
                nc.vector.tensor_scalar(
                    w32[:], w32[:], shv[:], 0x0F0F0F0F,
                    mybir.AluOpType.logical_shift_right,
                    mybir.AluOpType.bitwise_and,
                )
                # (nibble - 8) cast to fp16 on ACT
                if kt < fp16_kt:
                    dst = wt_pool.tile([P, nc_n], mybir.dt.float16, tag=f"wt{kt}")
                    tmp = dst
                else:
                    pr, sl = divmod(kt - fp16_kt, 2)
                    if sl == 0:
                        wt8_tiles[pr] = wt8_pool.tile(
                            [P, 2, nc_n], mybir.dt.float8e4, tag=f"wt8_{pr}",
                            name=f"wt8_{pr}",
                        )
                    tmp = tmp_pool.tile([P, nc_n], mybir.dt.float16, tag="tmp16")
                nc.scalar.activation(
                    tmp[:], wsb[:], mybir.ActivationFunctionType.Identity,
                    bias=-8.0, scale=1.0,
                )
                # replicate the scale row across partitions on the PE
                # (contraction-1 matmul) and fold it in on DVE straight from
                # PSUM; fp8 groups downconvert to f8e4 (RNE) in the same op
                for n0, w in n_splits:
                    psb = psb_pool.tile([P, MM_FREE], mybir.dt.float32,
                                        tag="psb", name=f"psb_{kt}_{n0}")
                    nc.tensor.matmul(
                        psb[:, :w], ones[:], srow[:, n0 : n0 + w],
                        start=True, stop=True,
                    )
                    if kt < fp16_kt:
                        nc.vector.tensor_tensor(
                            tmp[:, n0 : n0 + w], tmp[:, n0 : n0 + w],
                            psb[:, :w], mybir.AluOpType.mult,
                        )
                    else:
                        nc.vector.tensor_tensor(
                            wt8_tiles[pr][:, sl, n0 : n0 + w],
                            tmp[:, n0 : n0 + w], psb[:, :w],
                            mybir.AluOpType.mult,
                        )
                if kt < fp16_kt:
                    wt_tiles.append(tmp)
                if kt == 0:
                    # x chunk 0 DMAs issue right after kt0's weight DMAs so
                    # the first matmul's inputs all land early
                    xt_tiles[0] = load_x_chunk(0)

            # ---- main loop: x^T chunks x W^T -> out rows ----
            for c0 in range(0, t, tok_chunk):
                if c0 in xt_tiles:
                    xsubs, x8c = xt_tiles.pop(c0)
                else:
                    xsubs, x8c = load_x_chunk(c0)
                for ts in range(tok_chunk // P):
                    psums = alloc_psums(c0 + ts * P)
                    for kt in range(fp16_kt):
                        emit_mms(psums, xsubs, ts, kt)
                    for pr in range(n_pairs):
                        emit_mms_fp8(psums, x8c, ts, pr)
                    emit_tail(psums, c0 + ts * P)
    return nc


# ---------------------------------------------------------------------------
def pack_inputs(x, weight_packed, scales, t=T, k=K, nc_n=NC_N, ncores=NCORES):
    """Host-side layout prep (transpose/permute/replicate only)."""
    x = np.asarray(x, dtype=np.float16).reshape(t, k)
    wp = np.asarray(weight_packed, dtype=np.uint8)
    sc = np.asarray(scales, dtype=np.float16)
    kt_n = k // P

    # xTp[kt*128 + par*64 + j, t] = x[t, kt*128 + 2j + par]
    xTp = np.ascontiguousarray(
        x.reshape(t, kt_n, 64, 2).transpose(1, 3, 2, 0).reshape(k, t)
    )

    in_maps = []
    for c in range(ncores):
        n0 = c * nc_n
        wpT = wp[n0 : n0 + nc_n].T  # [k/2, nc_n]
        v = wpT.reshape(kt_n, 64, nc_n)
        wpk = np.empty((kt_n, P, nc_n), dtype=np.uint8)
        wpk[:, 0:64] = v
        wpk[:, 64:P] = v
        sclT = sc[n0 : n0 + nc_n].T  # [kt_n, nc_n]
        scl = np.ascontiguousarray(sclT[:, None, :])  # [kt_n, 1, nc_n]
        in_maps.append({"xTp": xTp, "wpk": wpk, "scl": scl})
    return in_maps


def run(x, weight_packed, scales, trace=False):
    _install_ntff_shim()
    from concourse.bass_utils import run_bass_kernel_spmd

    nc = build_nc()
    _legalize_multi_waits(nc, max_waits=1)
    in_maps = pack_inputs(x, weight_packed, scales)
    # transient NRT device errors (NRT_EXEC_UNIT_UNRECOVERABLE) have been
    # observed to clear on retry; back off briefly between attempts.
    import time as _time

    last_exc = None
    for attempt in range(4):
        try:
            res = run_bass_kernel_spmd(
                nc, in_maps, core_ids=list(range(NCORES)), trace=trace
            )
            break
        except Exception as e:
            last_exc = e
            _time.sleep(15 * (attempt + 1))
    else:
        raise last_exc
    parts = [res.results[c]["out"] for c in range(NCORES)]
    full = np.concatenate(parts, axis=1).reshape(B, S, N)
    return full, res


def kernel(x, weight_packed, scales):
    full, _ = run(x, weight_packed, scales, trace=False)
    return full


if __name__ == "__main__":
    rng = np.random.default_rng(0)
    x = rng.standard_normal((B, S, K)).astype(np.float16)
    wp = rng.integers(0, 256, (N, K // 2)).astype(np.uint8)
    sc = (rng.random((N, K // KT)).astype(np.float16) * np.float16(0.1))
    out = kernel(x, wp, sc)
    print(out.shape, out.dtype)

